# revision 23
# baseline (speedup 1.0000x reference)
"""Trainium2 Bass kernel for nn_AttentionHead.

Computation (per batch b):
    Q = Wq @ x_b, K = Wk @ x_b, V = Wv @ x_b        (x_b: [C=256, N=4096])
    S = Q^T K   [N, N];  A = softmax_k(S)
    out_b = V @ A^T                                  ([VC=128, N])

Sharding: 8 cores = 4 batches x 2 query-halves. Each core computes K/V^T for
its full batch and Q for its 2048-query half, processed as 4 passes of 512
queries; a flash-style loop over 32 key chunks of 128 never materializes the
full [4096, 4096] affinity.

Numerics: host casts x and weights to fp16; matmuls run on the 16-bit PE
path with fp32 PSUM accumulation. exp tiles are bf16 (logits reach ~19, so
fp16 would overflow after exp). Softmax denominators: bf16 DVE chain-adds
down to 2 partial tiles per pass; final cross-partition reduction and the
normalization happen on the host during unshard.

Schedule: the exp stream is the wall (ACT processes 8.4M elements/core at
1 elem/cycle/lane @1.2GHz ~= 54.6us + per-instruction overhead), so exps are
1536 wide: key chunks are grouped (2,3,3,...,3) per 512-query pass, QK
chunks land in a 2x[128,1536] PSUM ping-pong (6 banks), one ACTIVATE per
group. V^T is produced by X-bar DMA transpose on the sync queue (V projected
512-wide like K), keeping the PE to QK + AV + 24 projection matmuls. AV
matmuls accumulate into a 1-bank pc and drain lazily (deque) so the PE
prioritizes QK, which feeds ACT. Dummy matmuls at t~0 warm the PE HAM clock
gate (1.2->2.4GHz) before the head. Input DMA is split cc0->sync /
cc1->gpsimd in consumption order.
"""

from collections import deque

import numpy as np

B, C, VC, H, W = 4, 256, 128, 64, 64
N = H * W            # keys per batch
MQ = N // 2          # queries per core
QP = 512             # queries per pass
NP = MQ // QP        # passes
KC = N // 128        # key chunks of 128

# key-chunk groups per pass: (0,1) then ten groups of 3
GROUPS = [(0, 1)] + [tuple(range(2 + 3 * i, 5 + 3 * i)) for i in range(10)]

DMA_TRANSPOSE_VT = True   # X-bar transpose for V^T (else PE quad matmuls)

_cached_nc = None


def _build():
    from contextlib import ExitStack

    import concourse.bacc as bacc
    import concourse.mybir as mybir
    import concourse.tile as tile

    f32 = mybir.dt.float32
    f16 = mybir.dt.float16
    bf16 = mybir.dt.bfloat16
    Exp = mybir.ActivationFunctionType.Exp

    nc = bacc.Bacc("TRN2", target_bir_lowering=False, debug=False, num_devices=8)

    # Host-packed inputs: every DMA source is a contiguous [128, cols] block.
    #   wz: both C-halves of [Wq^T | Wk^T | Wv^T]
    #   xk1: x cols 0:1024 as 4 blocks (2 col-spans x 2 C-halves)
    #   xk2: x cols 1024:4096 as 6 blocks of 1024 (3 spans x 2 C-halves)
    wz_d = nc.dram_tensor("wz", [2, 128, 3 * VC], f16, kind="ExternalInput")
    xk1_d = nc.dram_tensor("xk1", [4, 128, 512], f16, kind="ExternalInput")
    xk2_d = nc.dram_tensor("xk2", [6, 128, 1024], f16, kind="ExternalInput")
    oc_d = nc.dram_tensor("oc", [NP, 128, QP], bf16, kind="ExternalOutput")
    # per pass: [0:1536] = tree-summed partial, [1536:2560] = ragged (chunks 0,1)
    oss_d = nc.dram_tensor("oss", [NP, 128, 2560], bf16, kind="ExternalOutput")

    with tile.TileContext(nc) as tc, ExitStack() as ctx:
        persist = ctx.enter_context(tc.tile_pool(name="persist", bufs=1))
        wpool = ctx.enter_context(tc.tile_pool(name="w", bufs=1))
        xp = ctx.enter_context(tc.tile_pool(name="xp", bufs=1))

        wall_t = [
            wpool.tile([128, 3 * VC], f16, tag=f"wall{cc}", name=f"wall{cc}")
            for cc in range(2)
        ]
        _woff = {"wq": 0, "wk": VC, "wv": 2 * VC}
        wts = {
            (nm, cc): wall_t[cc][:, off : off + VC]
            for nm, off in _woff.items()
            for cc in range(2)
        }

        K_t = persist.tile([128, N], f16, tag="K")
        Q_t = persist.tile([128, MQ], f16, tag="Q")
        V_t = persist.tile([128, N], bf16, tag="V")
        VT = persist.tile([128, KC * 128], bf16, tag="VT")
        scratch = persist.tile([128, 512], f16, tag="scratch")

        xk_t = [
            xp.tile([128, N], f16, tag=f"xk{cc}", name=f"xk{cc}") for cc in range(2)
        ]
        # Input DMA: cc0 pieces on sync, cc1 pieces on gpsimd, both queues in
        # consumption order; per-engine dependency windows bound in-flight
        # pieces so early pieces finish early.
        from concourse.tile_rust import add_dep_helper

        _dmas = {}

        def _issue(eng, ename, dst_ap, src_ap):
            lst = _dmas.setdefault(ename, [])
            ins = eng.dma_start(dst_ap, src_ap)
            if len(lst) >= 4:
                add_dep_helper(ins.ins, lst[-4].ins, reason="dma window")
            lst.append(ins)

        for cc, (eng, ename) in enumerate(((nc.sync, "s"), (nc.gpsimd, "g"))):
            _issue(eng, ename, wall_t[cc][:], wz_d[cc])
            _issue(eng, ename, xk_t[cc][:, 0:512], xk1_d[0 + cc])
            _issue(eng, ename, xk_t[cc][:, 512:1024], xk1_d[2 + cc])
            for blk in range(3):
                _issue(
                    eng, ename,
                    xk_t[cc][:, 1024 + blk * 1024 : 2048 + blk * 1024],
                    xk2_d[2 * blk + cc],
                )

        # DMA-arrival floors (ms): don't schedule projection work before its
        # piece can have landed, so the in-order PE queue never blocks.
        QF0, KF0 = 0.0090, 0.0092
        TILE_FLOOR = {0: 0.0100, 1: 0.0108, 2: 0.0134, 3: 0.0136, 4: 0.0155,
                      5: 0.0157, 6: 0.0176, 7: 0.0178}
        QF = {1: 0.0110, 2: 0.0112, 3: 0.0114}

        spool = ctx.enter_context(tc.tile_pool(name="spool", bufs=2, space="PSUM"))
        pcpool = ctx.enter_context(tc.tile_pool(name="pcpool", bufs=1, space="PSUM"))
        pjpool = ctx.enter_context(tc.tile_pool(name="pjpool", bufs=1, space="PSUM"))


        # 8 dummy matmuls warm the PE clock gate (1.2->2.4GHz) in the
        # 6.5-10.4us DMA-wait head window; garbage output lands in the pj
        # bank, which Q0's start=True matmul clears. More would delay the
        # head: the preamble holds PE until ~6.5us and data lands ~10.4.
        warm = pjpool.tile([128, 512], f32, tag="pj", name="warm")
        nc.vector.memset(scratch[:, 0:512], 0.0)
        for _ in range(8):
            nc.tensor.matmul(
                warm[:, 0:512], scratch[:, 0:128], scratch[:, 0:512],
                start=True, stop=True,
            )

        def emit_proj_tile(pool, tag, dst, wnm, t, copy_eng):
            ps = pool.tile([128, 512], f32, tag=tag, name="ps")
            for cc in range(2):
                nc.tensor.matmul(
                    ps[:, 0:512],
                    wts[(wnm, cc)][:],
                    xk_t[cc][:, t * 512 : (t + 1) * 512],
                    start=(cc == 0),
                    stop=(cc == 1),
                )
            copy_eng(dst[:, t * 512 : (t + 1) * 512], ps[:, 0:512])

        def emit_v_tile(pool, tag, t, copy_eng):
            # project V tile then X-bar transpose it into VT on the sync queue
            emit_proj_tile(pool, tag, V_t, "wv", t, copy_eng)
            src = V_t[:, t * 512 : (t + 1) * 512]
            dst = VT[:, t * 512 : (t + 1) * 512].rearrange(
                "p (a c) -> p a c", a=4
            )
            nc.sync.dma_start_transpose(dst, src)

        def emit_vt_quad(pool, tag, q, copy_eng):
            # fallback: V^T blocks via PE (x_block stationary transposes)
            tp = pool.tile([128, 512], f32, tag=tag, name="tp")
            for jj in range(4):
                j = 4 * q + jj
                for cc in range(2):
                    nc.tensor.matmul(
                        tp[:, jj * 128 : (jj + 1) * 128],
                        xk_t[cc][:, j * 128 : (j + 1) * 128],
                        wts[("wv", cc)][:],
                        start=(cc == 0),
                        stop=(cc == 1),
                    )
            copy_eng(VT[:, q * 512 : (q + 1) * 512], tp[:, 0:512])

        # in-loop projection schedule: {group idx emitted at: units}.
        # K tiles have hard deadlines (QK emission); V quads ("Vq", PE
        # transpose-by-matmul) serve early AV chunks; V tiles 4-7 ("Vp")
        # project early and X-bar-transpose on the sync queue, whose
        # serialized transfers land well before their late AV consumption.
        PROJ = {
            1: [("K", 2)],
            2: [("K", 3), ("Vp", 1)],
            3: [("K", 4), ("Vp", 2)],
            4: [("Vp", 3)],
            5: [("K", 5), ("Vp", 4)],
            6: [("K", 6), ("Vp", 5)],
            7: [("K", 7), ("Vp", 6)],
            8: [("Q", 1), ("Vp", 7)],
            11: [("Q", 2)],
            13: [("Q", 3)],
        }

        with (
            tc.tile_pool(name="epool", bufs=14) as epool,
            tc.tile_pool(name="accp", bufs=2) as accp,
            tc.tile_pool(name="opool", bufs=2) as opool,
        ):
            # flattened (pass, group) sequence
            seq = [(p, g) for p in range(NP) for g in range(len(GROUPS))]
            qk_done = {}

            def emit_qk(p, g):
                chunks = GROUPS[g]
                ps = spool.tile([128, 1536], f32, tag="ps", name="ps")
                for i, c in enumerate(chunks):
                    nc.tensor.matmul(
                        ps[:, i * 512 : (i + 1) * 512],
                        K_t[:, c * 128 : (c + 1) * 128],
                        Q_t[:, p * QP : (p + 1) * QP],
                        start=True,
                        stop=True,
                    )
                qk_done[(p, g)] = ps

            # lazy AV state
            pend_av = deque()           # (p, c, es, islice)
            pc_t = {"t": None}

            def drain_av(n):
                for _ in range(n):
                    if not pend_av:
                        return
                    p, c, es, isl = pend_av.popleft()
                    if c == 0:
                        pc_t["t"] = pcpool.tile(
                            [128, QP], f32, tag="pc", name="pc"
                        )
                    nc.tensor.matmul(
                        pc_t["t"][:, 0:QP],
                        VT[:, c * 128 : (c + 1) * 128],
                        es[:, isl * 512 : (isl + 1) * 512],
                        start=(c == 0),
                        stop=(c == KC - 1),
                    )
                    if c == KC - 1:
                        # drain pc: copy + DMA, split across queues
                        so = opool.tile([128, QP], bf16, tag="so", name="so")
                        nc.vector.tensor_copy(so[:, 0:QP], pc_t["t"][:, 0:QP])
                        eng = nc.gpsimd if p % 2 == 0 else nc.sync
                        eng.dma_start(oc_d[p], so[:, 0:QP])

            # ---- head: Q0 (pj) and K0 (pc) in parallel chains, copies split
            # DVE/ACT; QK(group 0) right behind them so the first exp is early.
            with tc.tile_wait_until(QF0):
                emit_proj_tile(pjpool, "pj", Q_t, "wq", 0, nc.vector.tensor_copy)
            with tc.tile_wait_until(KF0):
                emit_proj_tile(pcpool, "pc", K_t, "wk", 0, nc.scalar.copy)
            emit_qk(0, 0)
            with tc.tile_wait_until(TILE_FLOOR[0]):
                emit_vt_quad(pjpool, "pj", 0, nc.vector.tensor_copy)
            with tc.tile_wait_until(TILE_FLOOR[1]):
                emit_proj_tile(pcpool, "pc", K_t, "wk", 1, nc.scalar.copy)

            # deferred softmax-sum adds: (pass, kind, es/None); drained into
            # DVE slack after pass 0 so proj copies aren't queued behind them
            pend_sum = deque()
            acc = {"t": None, "first": None}

            def drain_sums(n):
                for _ in range(n):
                    if not pend_sum:
                        return
                    sp_, kind, es_t = pend_sum.popleft()
                    if kind == "first":
                        acc["first"] = es_t
                    elif kind == "second":
                        at = accp.tile([128, 1536], bf16, tag="acc", name="at")
                        acc["t"] = at
                        nc.vector.tensor_add(
                            at[:, 0:1536], acc["first"][:, 0:1536],
                            es_t[:, 0:1536],
                        )
                    else:
                        nc.vector.tensor_add(
                            acc["t"][:, 0:1536], acc["t"][:, 0:1536],
                            es_t[:, 0:1536],
                        )
                    if kind == "last":
                        eng = nc.sync if sp_ % 2 == 0 else nc.gpsimd
                        eng.dma_start(oss_d[sp_, :, 0:1536], acc["t"][:, 0:1536])

            def emit_unit(kind, idx):
                if kind == "K":
                    with tc.tile_wait_until(TILE_FLOOR[idx]):
                        emit_proj_tile(
                            pjpool, "pj", K_t, "wk", idx, nc.vector.tensor_copy
                        )
                elif kind == "Vq":
                    with tc.tile_wait_until(TILE_FLOOR[idx]):
                        emit_vt_quad(pjpool, "pj", idx, nc.vector.tensor_copy)
                elif kind == "Vp":
                    with tc.tile_wait_until(TILE_FLOOR[idx]):
                        emit_v_tile(pjpool, "pj", idx, nc.vector.tensor_copy)
                else:
                    with tc.tile_wait_until(QF[idx]):
                        emit_proj_tile(
                            pjpool, "pj", Q_t, "wq", idx, nc.vector.tensor_copy
                        )

            for gi, (p, g) in enumerate(seq):
                chunks = GROUPS[g]
                w = 512 * len(chunks)
                ps = qk_done.pop((p, g))
                es = epool.tile([128, 1536], bf16, tag="es", name="es")
                nc.scalar.activation(es[:, 0:w], ps[:, 0:w], Exp)
                # first proj unit before next QK; the second after, so its
                # WAR wait on the shared pj bank overlaps the QK matmuls.
                units = PROJ.get(gi, ())
                if units:
                    emit_unit(*units[0])
                if gi + 1 < len(seq):
                    emit_qk(*seq[gi + 1])
                for kind, idx in units[1:]:
                    emit_unit(kind, idx)
                # queue this group's AV work; drain with priority to QK/proj
                for i, c in enumerate(chunks):
                    pend_av.append((p, c, es, i))
                if p == 0:
                    cap = 0 if units else (2 if gi == 0 else 4)
                else:
                    cap = 2 if units else (4 if len(pend_av) > 6 else 3)
                drain_av(cap)
                # softmax partial sums: ragged group ships as-is, full
                # groups chain-added into one [128,1536] accumulator
                # (deferred past pass 0 so copies aren't queued behind adds).
                if g == 0:
                    eng2 = nc.gpsimd if p % 2 == 0 else nc.sync
                    eng2.dma_start(oss_d[p, :, 1536:2560], es[:, 0:1024])
                elif g == 1:
                    pend_sum.append((p, "first", es))
                elif g == 2:
                    pend_sum.append((p, "second", es))
                elif g == len(GROUPS) - 1:
                    pend_sum.append((p, "last", es))
                else:
                    pend_sum.append((p, "mid", es))
                drain_sums(0 if gi < 9 else 2)
            drain_av(len(pend_av))
            drain_sums(len(pend_sum))
    nc.compile()
    return nc


def make_in_maps(x, Wq, Wk, Wv):
    x = np.asarray(x, dtype=np.float32).reshape(B, C, N).astype(np.float16)
    wz = np.concatenate(
        [np.asarray(w, np.float32).T for w in (Wq, Wk, Wv)], axis=1
    ).astype(np.float16)
    wt = {"wz": np.ascontiguousarray(wz.reshape(2, 128, 3 * VC))}

    def blocks(m, spans):
        # pack [cc-half, col-span] blocks of a [C, *] matrix contiguously
        return np.stack(
            [m[cc * 128 : (cc + 1) * 128, c0:c1] for c0, c1 in spans for cc in (0, 1)]
        )

    in_maps = []
    for core in range(8):
        b, h = core // 2, core % 2
        # key order is free (softmax sums over all keys): pack this
        # core's query half first so Q projects from xk at offset 0
        xc = x[b] if h == 0 else np.concatenate(
            [x[b][:, MQ:], x[b][:, :MQ]], axis=1
        )
        in_maps.append(
            {
                "xk1": blocks(xc, [(0, 512), (512, 1024)]),
                "xk2": blocks(
                    xc, [(1024, 2048), (2048, 3072), (3072, 4096)]
                ),
                **wt,
            }
        )
    return in_maps


def assemble_core(oc, oss):
    """[NP,128,QP] bf16 oc + [NP,128,2560] bf16 oss -> [128, MQ] fp32."""
    oc = np.asarray(oc, dtype=np.float32)
    oss = np.asarray(oss, dtype=np.float32)
    out = np.empty((VC, MQ), dtype=np.float32)
    for p in range(NP):
        main = oss[p, :, 0:1536].reshape(128, 3, QP).sum(axis=(0, 1))
        rag = oss[p, :, 1536:2560].reshape(128, 2, QP).sum(axis=(0, 1))
        out[:, p * QP : (p + 1) * QP] = oc[p] / (main + rag)[None, :]
    return out


def assemble_output(results):
    out = np.empty((B, VC, N), dtype=np.float32)
    for core, r in enumerate(results):
        b, h = core // 2, core % 2
        out[b, :, h * MQ : (h + 1) * MQ] = assemble_core(r["oc"], r["oss"])
    return out.reshape(B, VC, H, W)


def _results_sane(results):
    for r in results:
        oc = np.asarray(r["oc"], dtype=np.float32)
        oss = np.asarray(r["oss"], dtype=np.float32)
        if not (np.isfinite(oc).all() and np.isfinite(oss).all()):
            return False
        sums = (
            oss[:, :, 0:1536].reshape(NP, 128, 3, QP).sum(axis=(1, 2))
            + oss[:, :, 1536:2560].reshape(NP, 128, 2, QP).sum(axis=(1, 2))
        )
        if sums.min() <= 0.0:      # softmax denominators
            return False
    return True


def kernel(x, Wq, Wk, Wv):
    global _cached_nc
    from concourse.bass_utils import run_bass_kernel_spmd

    if _cached_nc is None:
        _cached_nc = _build()
    in_maps = make_in_maps(x, Wq, Wk, Wv)
    results = None
    for attempt in range(3):
        try:
            res = run_bass_kernel_spmd(
                _cached_nc, in_maps, core_ids=list(range(8))
            )
        except Exception:
            if attempt == 2:
                raise
            continue
        results = res.results
        if _results_sane(results):
            break
    return assemble_output(results)
